# revision 7
# baseline (speedup 1.0000x reference)
"""AttentionBlock kernel for 8 Trainium2 NeuronCores.

Sharding: one (batch, head) pair per core (B=2 x H=4 = 8 cores).
Per core, for its (b, h):
    qT/kT = (w_q/k^T @ x_b) + bias            [64, S] fp16  (S pad 2816)
    scores S^T[j, i] = sum_d k[j,d] q[i,d]    fp16 matmuls, fp32 PSUM
    P = exp(S^T * 0.125 - 3)  -> fp8e4m3, two engine lanes:
        even j-tile pairs: ScalarE native exp (fp8 out)
        odd  j-tile pairs: DVE Schraudolph: u8 = rint(s*(1/ln16)+21.03),
                           bitcast to e4m3 (exact 2^x encode +-3% ripple;
                           softmax normalization cancels the mean bias)
    resT[d, i] = sum_j v8[j, d] P[j, i]       fp8 DoubleRow matmuls: two
        j-tiles (256 rows) per instruction at 0.5 cycles/col; v8 is fp8
        [128, 2, 128] with col 64 = ones (softmax denominator l in row 64),
        cols 65..127 zero (dual-fp8 ldweights requires M=128).
    outT[c, i] = sum_d w_out[d, c] resT[d, i] fp16, fp32 PSUM -> fp16 SBUF
Host: out_b = sum_h (outT / l + (b_v @ w_out_h)) + b_out + x_b.

Engine budget per core (~40us): PE 96K cycles @2.4GHz (scores fp16 are
60K of it; fp8 DR scores were tried in design but the PSUM-drain cost of
plane-split q/k on ScalarE/DVE outweighs the PE saving since Pool cannot
access PSUM). ScalarE+DVE split exp + all PSUM drains (~76K lane-elems).
"""

import numpy as np

C = 256
S = 2744
SP = 2816  # 22 * 128
H = 4
DK = 64
NT = 22  # j tiles of 128
NG = 11  # groups of 2 j-tiles
SVALID_LAST = S - 21 * 128  # 56 valid rows in last j-tile

IBLOCKS = [(0, 512), (512, 512), (1024, 512), (1536, 512), (2048, 512), (2560, 184)]
SBLOCKS = [(0, 512), (512, 512), (1024, 512), (1536, 512), (2048, 512), (2560, 256)]

LN2 = float(np.log(2.0))
# Schraudolph byte encode: u8 = rint(s * SCHRAU_SCALE + SCHRAU_BIAS)
SCHRAU_SCALE = 0.125 * 8.0 / LN2
SCHRAU_BIAS = 56.0 - 24.0 / LN2 - 0.344  # -0.344 centers ripple vs exact lane

_NC = None


def _build():
    from contextlib import ExitStack

    import concourse.bacc as bacc
    import concourse.tile as tile
    from concourse import mybir

    f32 = mybir.dt.float32
    f16 = mybir.dt.float16
    f8 = mybir.dt.float8e4
    u8 = mybir.dt.uint8
    Exp = mybir.ActivationFunctionType.Exp
    DR = mybir.MatmulPerfMode.DoubleRow
    Mult = mybir.AluOpType.mult
    Add = mybir.AluOpType.add

    nc = bacc.Bacc("TRN2", target_bir_lowering=False)

    xT = nc.dram_tensor("xT", [C, S], f16, kind="ExternalInput")
    wq = nc.dram_tensor("wq", [C, DK], f16, kind="ExternalInput")
    wk = nc.dram_tensor("wk", [C, DK], f16, kind="ExternalInput")
    wv = nc.dram_tensor("wv", [C, DK], f16, kind="ExternalInput")
    bqk = nc.dram_tensor("bqk", [DK, 2], f32, kind="ExternalInput")
    wo = nc.dram_tensor("wo", [DK, C], f16, kind="ExternalInput")
    ones_tail = nc.dram_tensor("ones_tail", [128, 1], f8, kind="ExternalInput")

    out = nc.dram_tensor("out", [C, S], f16, kind="ExternalOutput")
    lsum = nc.dram_tensor("lsum", [1, S], f16, kind="ExternalOutput")

    with tile.TileContext(nc) as tc, ExitStack() as ctx:
        consts = ctx.enter_context(tc.tile_pool(name="consts", bufs=1))
        big = ctx.enter_context(tc.tile_pool(name="big", bufs=1))
        expp = ctx.enter_context(tc.tile_pool(name="expp", bufs=4))
        resp = ctx.enter_context(tc.tile_pool(name="resp", bufs=2))
        # PSUM: scp 2x[128,1024](4 banks) + qop 2x[128,512](2) + pvp 2x(2)
        scp = ctx.enter_context(tc.tile_pool(name="scp", bufs=2, space="PSUM"))
        qop = ctx.enter_context(tc.tile_pool(name="qop", bufs=2, space="PSUM"))
        pvp = ctx.enter_context(tc.tile_pool(name="pvp", bufs=2, space="PSUM"))

        # ---- weights / constants ----
        w_sb = consts.tile([128, 2, 3 * DK], f16)
        for idx, w_dram in enumerate((wq, wk, wv)):
            nc.gpsimd.dma_start(
                out=w_sb[:, :, idx * DK : (idx + 1) * DK],
                in_=w_dram.rearrange("(c p) d -> p c d", p=128),
            )

        def wslice(idx, cc):
            return w_sb[:, cc, idx * DK : (idx + 1) * DK]

        wo_sb = consts.tile([DK, C], f16)
        nc.gpsimd.dma_start(out=wo_sb, in_=wo[:, :])
        b_sb = consts.tile([DK, 2], f32)
        nc.gpsimd.dma_start(out=b_sb, in_=bqk[:, :])
        ebias_sb = consts.tile([128, 1], f32)
        nc.vector.memset(ebias_sb, -3.0)

        # ---- x in SBUF ----
        x_sb = big.tile([128, 2, SP], f16)
        nc.vector.memset(x_sb[:, :, S:SP], 0.0)
        nc.sync.dma_start(
            out=x_sb[:, :, :S], in_=xT.rearrange("(c p) s -> p c s", p=128)
        )

        # ---- persistent big tiles ----
        qT_sb = big.tile([DK, SP], f16)
        kT_sb = big.tile([DK, SP], f16)
        # v8: [j-in-tile, group, plane(u), 128]: cols 0:64 v, 64 ones, 65: zero
        v8_sb = big.tile([128, NG, 2, 128], f8)
        nc.vector.memset(v8_sb, 0.0)
        nc.vector.memset(v8_sb[:, :, :, DK : DK + 1], 1.0)
        # last j-tile (group 10, plane 1) has only SVALID_LAST valid rows;
        # partition-base-56 memset is illegal, so DMA a host mask instead
        nc.gpsimd.dma_start(
            out=v8_sb[:, NG - 1, 1, DK : DK + 1], in_=ones_tail[:, :]
        )
        out_sb = big.tile([128, 2, S], f16)

        # ---- q/k projections (all blocks upfront) ----
        Identity = mybir.ActivationFunctionType.Identity

        def qk_chunk(sb, widx, dst, bias, eng):
            off, w = SBLOCKS[sb]
            ps = qop.tile([128, 512], f32, tag="q", name="psqk")
            for cc in range(2):
                nc.tensor.matmul(
                    ps[:DK, :w],
                    lhsT=wslice(widx, cc),
                    rhs=x_sb[:, cc, off : off + w],
                    start=(cc == 0),
                    stop=(cc == 1),
                )
            if eng is nc.scalar:
                nc.scalar.activation(
                    out=dst[:, off : off + w],
                    in_=ps[:DK, :w],
                    func=Identity,
                    bias=bias,
                )
            else:
                eng.tensor_scalar(dst[:, off : off + w], ps[:DK, :w], bias, None, Add)

        for sb in range(6):
            qk_chunk(sb, 1, kT_sb, b_sb[:, 1:2], nc.vector if sb % 2 else nc.scalar)
        for sb in range(6):
            qk_chunk(sb, 0, qT_sb, b_sb[:, 0:1], nc.vector if sb % 2 else nc.scalar)

        # ---- v projection chunk (pair of j-tiles -> one v8 group) ----
        def v_chunk(g):
            psv = qop.tile([128, 512], f32, tag="q", name="psv")
            pv3 = psv[:, :128].rearrange("p (u d) -> p u d", u=2)
            for u in range(2):
                t = 2 * g + u
                for cc in range(2):
                    nc.tensor.matmul(
                        pv3[:, u, :],
                        lhsT=x_sb[:, cc, t * 128 : (t + 1) * 128],
                        rhs=wslice(2, cc),
                        start=(cc == 0),
                        stop=(cc == 1),
                    )
            nc.vector.tensor_copy(v8_sb[:, g, :, :DK], pv3)

        # ---- main attention loop ----
        def emit_pv(pv, pex, pg, iw):
            nc.tensor.matmul(
                pv[:, :iw],
                lhsT=v8_sb[:, pg, :, :],
                rhs=pex[:, :, :iw],
                start=(pg == 0),
                stop=(pg == NG - 1),
                perf_mode=DR,
            )

        pending_tail = None
        for ibi, (ioff, iw) in enumerate(IBLOCKS):
            pv = pvp.tile([128, 512], f32, tag="pv", name="pv")
            pending_pv = []
            for g in range(NG):
                if ibi == 0:
                    v_chunk(g)
                sc = scp.tile([128, 1024], f32, tag="sc", name="sc")
                sc3 = sc.rearrange("p (b w) -> p b w", b=2)[:, :, :iw]
                for u in range(2):
                    t = 2 * g + u
                    nc.tensor.matmul(
                        sc3[:, u, :],
                        lhsT=kT_sb[:, t * 128 : (t + 1) * 128],
                        rhs=qT_sb[:, ioff : ioff + iw],
                        start=True,
                        stop=True,
                    )
                ex = expp.tile([128, 2, 512], f8, tag="ex", name="ex")
                ex3 = ex[:, :, :iw]
                if g % 2 == 0:
                    nc.scalar.activation(
                        out=ex3, in_=sc3, func=Exp, bias=ebias_sb, scale=0.125
                    )
                else:
                    nc.vector.tensor_scalar(
                        ex3.bitcast(u8), sc3, SCHRAU_SCALE, SCHRAU_BIAS, Mult, Add
                    )
                if g == 1 and pending_tail is not None:
                    pending_tail()
                    pending_tail = None
                pending_pv.append((ex, g))
                if len(pending_pv) > 1:
                    pex, pg = pending_pv.pop(0)
                    emit_pv(pv, pex, pg, iw)
            for pex, pg in pending_pv:
                emit_pv(pv, pex, pg, iw)
            res_sb = resp.tile([DK + 1, 512], f16, tag="res", name="res_sb")
            nc.scalar.copy(res_sb[:, :iw], pv[: DK + 1, :iw])
            nc.sync.dma_start(
                out=lsum[0:1, ioff : ioff + iw], in_=res_sb[DK : DK + 1, :iw]
            )

            def tail(ioff=ioff, iw=iw, res_sb=res_sb):
                for cc in range(2):
                    po = qop.tile([128, 512], f32, tag="q", name="po")
                    nc.tensor.matmul(
                        po[:, :iw],
                        lhsT=wo_sb[:, cc * 128 : (cc + 1) * 128],
                        rhs=res_sb[:DK, :iw],
                        start=True,
                        stop=True,
                    )
                    eng = nc.scalar if cc == 0 else nc.vector
                    if cc == 0:
                        nc.scalar.copy(out_sb[:, cc, ioff : ioff + iw], po[:, :iw])
                    else:
                        nc.vector.tensor_copy(
                            out_sb[:, cc, ioff : ioff + iw], po[:, :iw]
                        )

            pending_tail = tail
        pending_tail()
        nc.sync.dma_start(
            out=out.rearrange("(c p) s -> p c s", p=128), in_=out_sb
        )

    nc.compile()
    return nc


def _get_nc():
    global _NC
    if _NC is None:
        _NC = _build()
    return _NC


def _ones_tail():
    import ml_dtypes

    m = np.zeros((128, 1), dtype=ml_dtypes.float8_e4m3)
    m[:SVALID_LAST] = 1.0
    return m


def _make_in_maps(inputs):
    x = np.asarray(inputs["x"], dtype=np.float32)
    w_proj = np.asarray(inputs["w_proj"], dtype=np.float32)
    b_proj = np.asarray(inputs["b_proj"], dtype=np.float32)
    w_out = np.asarray(inputs["w_out"], dtype=np.float32)
    in_maps = []
    for core in range(8):
        b, h = divmod(core, H)
        base = h * 3 * DK
        in_maps.append(
            {
                "xT": np.ascontiguousarray(x[b].reshape(C, S).astype(np.float16)),
                "wq": np.ascontiguousarray(
                    w_proj[:, base : base + DK].astype(np.float16)
                ),
                "wk": np.ascontiguousarray(
                    w_proj[:, base + DK : base + 2 * DK].astype(np.float16)
                ),
                "wv": np.ascontiguousarray(
                    w_proj[:, base + 2 * DK : base + 3 * DK].astype(np.float16)
                ),
                "bqk": np.ascontiguousarray(
                    np.stack(
                        [
                            b_proj[base : base + DK],
                            b_proj[base + DK : base + 2 * DK],
                        ],
                        axis=1,
                    ).astype(np.float32)
                ),
                "wo": np.ascontiguousarray(
                    w_out[h * DK : (h + 1) * DK, :].astype(np.float16)
                ),
                "ones_tail": _ones_tail(),
            }
        )
    return in_maps


def kernel(x, w_proj, b_proj, w_out, b_out):
    from concourse.bass_utils import run_bass_kernel_spmd

    x = np.asarray(x, dtype=np.float32)
    w_proj = np.asarray(w_proj, dtype=np.float32)
    b_proj = np.asarray(b_proj, dtype=np.float32)
    w_out = np.asarray(w_out, dtype=np.float32)
    b_out = np.asarray(b_out, dtype=np.float32)

    B = x.shape[0]
    nc = _get_nc()
    in_maps = _make_in_maps(
        {"x": x, "w_proj": w_proj, "b_proj": b_proj, "w_out": w_out, "b_out": b_out}
    )
    res = run_bass_kernel_spmd(nc, in_maps, list(range(8)))

    outs = np.zeros((B, C, S), dtype=np.float32)
    for b in range(B):
        acc = x[b].reshape(C, S).astype(np.float32) + b_out[:, None]
        for h in range(H):
            core = b * H + h
            dev_o = res.results[core]["out"].astype(np.float32)  # [C, S] unnorm
            l = res.results[core]["lsum"].astype(np.float32)  # [1, S]
            bv = b_proj[h * 3 * DK + 2 * DK : h * 3 * DK + 3 * DK]
            corr = bv @ w_out[h * DK : (h + 1) * DK, :]  # [C]
            acc = acc + dev_o / l + corr[:, None]
        outs[b] = acc
    return outs.reshape(B, C, 14, 14, 14)


# revision 12
# speedup vs baseline: 1.0346x; 1.0346x over previous
"""AttentionBlock kernel for 8 Trainium2 NeuronCores.

Sharding: one (batch, head) pair per core (B=2 x H=4 = 8 cores).
Per core, for its (b, h):
    qT/kT = (w_q/k^T @ x_b) + bias            [64, S] fp16  (S pad 2816)
    scores S^T[j, i] = sum_d k[j,d] q[i,d]    fp16 matmuls, fp32 PSUM
    P = exp(S^T * 0.125 - 3)  -> fp8e4m3, two engine lanes:
        even j-tile pairs: ScalarE native exp (fp8 out)
        odd  j-tile pairs: DVE Schraudolph: u8 = rint(s*(1/ln16)+21.03),
                           bitcast to e4m3 (exact 2^x encode +-3% ripple;
                           softmax normalization cancels the mean bias)
    resT[d, i] = sum_j v8[j, d] P[j, i]       fp8 DoubleRow matmuls: two
        j-tiles (256 rows) per instruction at 0.5 cycles/col; v8 is fp8
        [128, 2, 128] with col 64 = ones (softmax denominator l in row 64),
        cols 65..127 zero (dual-fp8 ldweights requires M=128).
    outT[c, i] = sum_d w_out[d, c] resT[d, i] fp16, fp32 PSUM -> fp16 SBUF
Host: out_b = sum_h (outT / l + (b_v @ w_out_h)) + b_out + x_b.

Engine budget per core (~40us): PE 96K cycles @2.4GHz (scores fp16 are
60K of it; fp8 DR scores were tried in design but the PSUM-drain cost of
plane-split q/k on ScalarE/DVE outweighs the PE saving since Pool cannot
access PSUM). ScalarE+DVE split exp + all PSUM drains (~76K lane-elems).
"""

import numpy as np

C = 256
S = 2744
SP = 2816  # 22 * 128
H = 4
DK = 64
NT = 22  # j tiles of 128
NG = 11  # groups of 2 j-tiles
SVALID_LAST = S - 21 * 128  # 56 valid rows in last j-tile

IBLOCKS = [(0, 512), (512, 512), (1024, 512), (1536, 512), (2048, 512), (2560, 184)]
SBLOCKS = [(0, 512), (512, 512), (1024, 512), (1536, 512), (2048, 512), (2560, 256)]

LN2 = float(np.log(2.0))
# Schraudolph byte encode: u8 = rint(s * SCHRAU_SCALE + SCHRAU_BIAS)
SCHRAU_SCALE = 0.125 * 8.0 / LN2
SCHRAU_BIAS = 56.0 - 24.0 / LN2 - 0.344  # -0.344 centers ripple vs exact lane

_NC = None


def _build():
    from contextlib import ExitStack

    import concourse.bacc as bacc
    import concourse.tile as tile
    from concourse import mybir

    f32 = mybir.dt.float32
    f16 = mybir.dt.float16
    f8 = mybir.dt.float8e4
    u8 = mybir.dt.uint8
    Exp = mybir.ActivationFunctionType.Exp
    DR = mybir.MatmulPerfMode.DoubleRow
    Mult = mybir.AluOpType.mult
    Add = mybir.AluOpType.add

    nc = bacc.Bacc("TRN2", target_bir_lowering=False)

    xT = nc.dram_tensor("xT", [C, S], f16, kind="ExternalInput")
    wq = nc.dram_tensor("wq", [C, DK], f16, kind="ExternalInput")
    wk = nc.dram_tensor("wk", [C, DK], f16, kind="ExternalInput")
    wv = nc.dram_tensor("wv", [C, DK], f16, kind="ExternalInput")
    bqk = nc.dram_tensor("bqk", [DK, 2], f32, kind="ExternalInput")
    wo = nc.dram_tensor("wo", [DK, C], f16, kind="ExternalInput")
    ones_tail = nc.dram_tensor("ones_tail", [128, 1], f8, kind="ExternalInput")

    out = nc.dram_tensor("out", [C, S], f16, kind="ExternalOutput")
    lsum = nc.dram_tensor("lsum", [1, S], f16, kind="ExternalOutput")

    with tile.TileContext(nc) as tc, ExitStack() as ctx:
        consts = ctx.enter_context(tc.tile_pool(name="consts", bufs=1))
        big = ctx.enter_context(tc.tile_pool(name="big", bufs=1))
        expp = ctx.enter_context(tc.tile_pool(name="expp", bufs=4))
        resp = ctx.enter_context(tc.tile_pool(name="resp", bufs=2))
        # PSUM 8 banks: scp 3x[128,1024]f32 (6 banks) shared by score pairs
        # AND the projection chunks; T-pool 1x[128,1024] (2 banks) holds
        # pv (cols 0:512), out-proj cc0 (512:1024), cc1 (0:512 after res).
        scp = ctx.enter_context(tc.tile_pool(name="scp", bufs=3, space="PSUM"))
        tp = ctx.enter_context(tc.tile_pool(name="tp", bufs=1, space="PSUM"))

        # ---- weights / constants ----
        w_sb = consts.tile([128, 2, 3 * DK], f16)
        for idx, w_dram in enumerate((wq, wk, wv)):
            nc.gpsimd.dma_start(
                out=w_sb[:, :, idx * DK : (idx + 1) * DK],
                in_=w_dram.rearrange("(c p) d -> p c d", p=128),
            )

        def wslice(idx, cc):
            return w_sb[:, cc, idx * DK : (idx + 1) * DK]

        wo_sb = consts.tile([DK, C], f16)
        nc.gpsimd.dma_start(out=wo_sb, in_=wo[:, :])
        b_sb = consts.tile([DK, 2], f32)
        nc.gpsimd.dma_start(out=b_sb, in_=bqk[:, :])
        ebias_sb = consts.tile([128, 1], f32)
        nc.vector.memset(ebias_sb, -3.0)

        # ---- x in SBUF ----
        x_sb = big.tile([128, 2, SP], f16)
        nc.vector.memset(x_sb[:, :, S:SP], 0.0)
        nc.sync.dma_start(
            out=x_sb[:, :, :S], in_=xT.rearrange("(c p) s -> p c s", p=128)
        )

        # ---- persistent big tiles ----
        qT_sb = big.tile([DK, SP], f16)
        kT_sb = big.tile([DK, SP], f16)
        # v8: [j-in-tile, group, plane(u), 128]: cols 0:64 v, 64 ones, 65+ 0
        v8_sb = big.tile([128, NG, 2, 128], f8)
        nc.vector.memset(v8_sb, 0.0)
        nc.vector.memset(v8_sb[:, :, :, DK : DK + 1], 1.0)
        # last j-tile (group 10, plane 1) has only SVALID_LAST valid rows;
        # partition-base-56 memset is illegal, so DMA a host mask instead
        nc.gpsimd.dma_start(
            out=v8_sb[:, NG - 1, 1, DK : DK + 1], in_=ones_tail[:, :]
        )
        out_sb = big.tile([128, 2, S], f16)

        Identity = mybir.ActivationFunctionType.Identity
        QBLOCKS = [(0, 1024), (1024, 1024), (2048, 768)]

        # ---- q/k projection chunk (1024-col blocks) ----
        def qk_chunk(qb, widx, dst, bias, eng):
            off, w = QBLOCKS[qb]
            ps = scp.tile([128, 1024], f32, tag="sc", name="psqk")
            for half in range(0, w, 512):
                hw_ = min(512, w - half)
                for cc in range(2):
                    nc.tensor.matmul(
                        ps[:DK, half : half + hw_],
                        lhsT=wslice(widx, cc),
                        rhs=x_sb[:, cc, off + half : off + half + hw_],
                        start=(cc == 0),
                        stop=(cc == 1),
                    )
            if eng is nc.scalar:
                nc.scalar.activation(
                    out=dst[:, off : off + w],
                    in_=ps[:DK, :w],
                    func=Identity,
                    bias=bias,
                )
            else:
                eng.tensor_scalar(
                    dst[:, off : off + w], ps[:DK, :w], bias, None,
                    mybir.AluOpType.add,
                )

        # ---- v projection chunk: 4 j-tiles -> v8 groups 2c, 2c+1 ----
        def v_chunk(c, eng):
            nt = min(4, NT - 4 * c)
            ps = scp.tile([128, 1024], f32, tag="sc", name="psv")
            pv4 = ps[:, : nt * DK].rearrange("p (t d) -> p t d", t=nt)
            for ti in range(nt):
                t = 4 * c + ti
                for cc in range(2):
                    nc.tensor.matmul(
                        pv4[:, ti, :],
                        lhsT=x_sb[:, cc, t * 128 : (t + 1) * 128],
                        rhs=wslice(2, cc),
                        start=(cc == 0),
                        stop=(cc == 1),
                    )
            dst = v8_sb[:, 2 * c : 2 * c + (nt + 1) // 2, :, :DK]
            eng.tensor_copy(dst, pv4)

        # phase A: k fully, q block 0, v chunk 0
        qk_chunk(0, 1, kT_sb, b_sb[:, 1:2], nc.scalar)
        qk_chunk(1, 1, kT_sb, b_sb[:, 1:2], nc.vector)
        qk_chunk(2, 1, kT_sb, b_sb[:, 1:2], nc.scalar)
        qk_chunk(0, 0, qT_sb, b_sb[:, 0:1], nc.vector)
        v_chunk(0, nc.vector)

        # chunks interleaved into iblock 0 (key: g -> emit fn)
        ib0_chunks = {
            1: lambda: v_chunk(1, nc.vector),
            2: lambda: qk_chunk(1, 0, qT_sb, b_sb[:, 0:1], nc.scalar),
            3: lambda: v_chunk(2, nc.vector),
            4: lambda: qk_chunk(2, 0, qT_sb, b_sb[:, 0:1], nc.scalar),
            5: lambda: v_chunk(3, nc.vector),
            7: lambda: v_chunk(4, nc.vector),
            9: lambda: v_chunk(5, nc.vector),
        }

        # ---- main attention loop ----
        def emit_pv(pv, pex, pg, iw):
            nc.tensor.matmul(
                pv[:, :iw],
                lhsT=v8_sb[:, pg, :, :],
                rhs=pex[:, :, :iw],
                start=(pg == 0),
                stop=(pg == NG - 1),
                perf_mode=DR,
            )

        pending_tail = None
        for ibi, (ioff, iw) in enumerate(IBLOCKS):
            T = None
            pv = None
            pending_pv = []
            for g in range(NG):
                if ibi == 0 and g in ib0_chunks:
                    ib0_chunks[g]()
                sc = scp.tile([128, 1024], f32, tag="sc", name="sc")
                sc3 = sc.rearrange("p (b w) -> p b w", b=2)[:, :, :iw]
                for u in range(2):
                    t = 2 * g + u
                    nc.tensor.matmul(
                        sc3[:, u, :],
                        lhsT=kT_sb[:, t * 128 : (t + 1) * 128],
                        rhs=qT_sb[:, ioff : ioff + iw],
                        start=True,
                        stop=True,
                    )
                ex = expp.tile([128, 2, 512], f8, tag="ex", name="ex")
                ex3 = ex[:, :, :iw]
                if (g + ibi) % 2 == 0:
                    nc.scalar.activation(
                        out=ex3, in_=sc3, func=Exp, bias=ebias_sb, scale=0.125
                    )
                else:
                    nc.vector.tensor_scalar(
                        ex3.bitcast(u8), sc3, SCHRAU_SCALE, SCHRAU_BIAS, Mult, Add
                    )
                if g == 0:
                    # old T's out-proj writes must be emitted BEFORE the
                    # new T allocation reuses the single tp buffer
                    if pending_tail is not None:
                        pending_tail()
                        pending_tail = None
                    T = tp.tile([128, 1024], f32, tag="T", name="T")
                    pv = T[:, 0:512]
                pending_pv.append((ex, g))
                if len(pending_pv) > 1:
                    pex, pg = pending_pv.pop(0)
                    emit_pv(pv, pex, pg, iw)
            for pex, pg in pending_pv:
                emit_pv(pv, pex, pg, iw)
            res_sb = resp.tile([DK + 1, 512], f16, tag="res", name="res_sb")
            nc.scalar.copy(res_sb[:, :iw], pv[: DK + 1, :iw])
            nc.sync.dma_start(
                out=lsum[0:1, ioff : ioff + iw], in_=res_sb[DK : DK + 1, :iw]
            )

            def tail(T=T, ioff=ioff, iw=iw, res_sb=res_sb):
                for cc in range(2):
                    po = T[:, 512:1024] if cc == 0 else T[:, 0:512]
                    nc.tensor.matmul(
                        po[:, :iw],
                        lhsT=wo_sb[:, cc * 128 : (cc + 1) * 128],
                        rhs=res_sb[:DK, :iw],
                        start=True,
                        stop=True,
                    )
                    if cc == 0:
                        nc.scalar.copy(out_sb[:, cc, ioff : ioff + iw], po[:, :iw])
                    else:
                        nc.vector.tensor_copy(
                            out_sb[:, cc, ioff : ioff + iw], po[:, :iw]
                        )

            pending_tail = tail
        pending_tail()
        nc.sync.dma_start(
            out=out.rearrange("(c p) s -> p c s", p=128), in_=out_sb
        )

    nc.compile()
    return nc


def _get_nc():
    global _NC
    if _NC is None:
        _NC = _build()
    return _NC


def _ones_tail():
    import ml_dtypes

    m = np.zeros((128, 1), dtype=ml_dtypes.float8_e4m3)
    m[:SVALID_LAST] = 1.0
    return m


def _make_in_maps(inputs):
    x = np.asarray(inputs["x"], dtype=np.float32)
    w_proj = np.asarray(inputs["w_proj"], dtype=np.float32)
    b_proj = np.asarray(inputs["b_proj"], dtype=np.float32)
    w_out = np.asarray(inputs["w_out"], dtype=np.float32)
    in_maps = []
    for core in range(8):
        b, h = divmod(core, H)
        base = h * 3 * DK
        in_maps.append(
            {
                "xT": np.ascontiguousarray(x[b].reshape(C, S).astype(np.float16)),
                "wq": np.ascontiguousarray(
                    w_proj[:, base : base + DK].astype(np.float16)
                ),
                "wk": np.ascontiguousarray(
                    w_proj[:, base + DK : base + 2 * DK].astype(np.float16)
                ),
                "wv": np.ascontiguousarray(
                    w_proj[:, base + 2 * DK : base + 3 * DK].astype(np.float16)
                ),
                "bqk": np.ascontiguousarray(
                    np.stack(
                        [
                            b_proj[base : base + DK],
                            b_proj[base + DK : base + 2 * DK],
                        ],
                        axis=1,
                    ).astype(np.float32)
                ),
                "wo": np.ascontiguousarray(
                    w_out[h * DK : (h + 1) * DK, :].astype(np.float16)
                ),
                "ones_tail": _ones_tail(),
            }
        )
    return in_maps


def kernel(x, w_proj, b_proj, w_out, b_out):
    from concourse.bass_utils import run_bass_kernel_spmd

    x = np.asarray(x, dtype=np.float32)
    w_proj = np.asarray(w_proj, dtype=np.float32)
    b_proj = np.asarray(b_proj, dtype=np.float32)
    w_out = np.asarray(w_out, dtype=np.float32)
    b_out = np.asarray(b_out, dtype=np.float32)

    B = x.shape[0]
    nc = _get_nc()
    in_maps = _make_in_maps(
        {"x": x, "w_proj": w_proj, "b_proj": b_proj, "w_out": w_out, "b_out": b_out}
    )
    res = run_bass_kernel_spmd(nc, in_maps, list(range(8)))

    outs = np.zeros((B, C, S), dtype=np.float32)
    for b in range(B):
        acc = x[b].reshape(C, S).astype(np.float32) + b_out[:, None]
        for h in range(H):
            core = b * H + h
            dev_o = res.results[core]["out"].astype(np.float32)  # [C, S] unnorm
            l = res.results[core]["lsum"].astype(np.float32)  # [1, S]
            bv = b_proj[h * 3 * DK + 2 * DK : h * 3 * DK + 3 * DK]
            corr = bv @ w_out[h * DK : (h + 1) * DK, :]  # [C]
            acc = acc + dev_o / l + corr[:, None]
        outs[b] = acc
    return outs.reshape(B, C, 14, 14, 14)


# revision 14
# speedup vs baseline: 1.1142x; 1.0769x over previous
"""AttentionBlock kernel for 8 Trainium2 NeuronCores.

Sharding: one (batch, head) pair per core (B=2 x H=4 = 8 cores).
Per core, for its (b, h):
    qT/kT = (w_q/k^T @ x_b) + bias            [64, S] fp16  (S pad 2816)
    scores S^T[j, i] = sum_d k[j,d] q[i,d]    fp16 matmuls, fp32 PSUM
    P = exp(S^T * 0.125 - 3)  -> fp8e4m3, two engine lanes:
        even j-tile pairs: ScalarE native exp (fp8 out)
        odd  j-tile pairs: DVE Schraudolph: u8 = rint(s*(1/ln16)+21.03),
                           bitcast to e4m3 (exact 2^x encode +-3% ripple;
                           softmax normalization cancels the mean bias)
    resT[d, i] = sum_j v8[j, d] P[j, i]       fp8 DoubleRow matmuls: two
        j-tiles (256 rows) per instruction at 0.5 cycles/col; v8 is fp8
        [128, 2, 128] with col 64 = ones (softmax denominator l in row 64),
        cols 65..127 zero (dual-fp8 ldweights requires M=128).
    outT[c, i] = sum_d w_out[d, c] resT[d, i] fp16, fp32 PSUM -> fp16 SBUF
Host: out_b = sum_h (outT / l + (b_v @ w_out_h)) + b_out + x_b.

Engine budget per core (~40us): PE 96K cycles @2.4GHz (scores fp16 are
60K of it; fp8 DR scores were tried in design but the PSUM-drain cost of
plane-split q/k on ScalarE/DVE outweighs the PE saving since Pool cannot
access PSUM). ScalarE+DVE split exp + all PSUM drains (~76K lane-elems).
"""

import numpy as np

C = 256
S = 2744
SP = 2816  # 22 * 128
H = 4
DK = 64
NT = 22  # j tiles of 128
NG = 11  # groups of 2 j-tiles
SVALID_LAST = S - 21 * 128  # 56 valid rows in last j-tile

IBLOCKS = [(0, 512), (512, 512), (1024, 512), (1536, 512), (2048, 512), (2560, 184)]
SBLOCKS = [(0, 512), (512, 512), (1024, 512), (1536, 512), (2048, 512), (2560, 256)]

LN2 = float(np.log(2.0))
# Schraudolph byte encode: u8 = rint(s * SCHRAU_SCALE + SCHRAU_BIAS)
SCHRAU_SCALE = 0.125 * 8.0 / LN2
SCHRAU_BIAS = 56.0 - 24.0 / LN2 - 0.344  # -0.344 centers ripple vs exact lane

_NC = None


def _build():
    from contextlib import ExitStack

    import concourse.bacc as bacc
    import concourse.tile as tile
    from concourse import mybir

    f32 = mybir.dt.float32
    f16 = mybir.dt.float16
    f8 = mybir.dt.float8e4
    u8 = mybir.dt.uint8
    Exp = mybir.ActivationFunctionType.Exp
    DR = mybir.MatmulPerfMode.DoubleRow
    Mult = mybir.AluOpType.mult
    Add = mybir.AluOpType.add

    nc = bacc.Bacc("TRN2", target_bir_lowering=False)

    xT = nc.dram_tensor("xT", [C, S], f16, kind="ExternalInput")
    wq = nc.dram_tensor("wq", [C, DK], f16, kind="ExternalInput")
    wk = nc.dram_tensor("wk", [C, DK], f16, kind="ExternalInput")
    wv = nc.dram_tensor("wv", [C, DK], f16, kind="ExternalInput")
    bqk = nc.dram_tensor("bqk", [DK, 2], f32, kind="ExternalInput")
    wo = nc.dram_tensor("wo", [DK, C], f16, kind="ExternalInput")
    ones_tail = nc.dram_tensor("ones_tail", [128, 1], f8, kind="ExternalInput")

    out = nc.dram_tensor("out", [C, S], f16, kind="ExternalOutput")
    lsum = nc.dram_tensor("lsum", [1, S], f16, kind="ExternalOutput")

    with tile.TileContext(nc) as tc, ExitStack() as ctx:
        consts = ctx.enter_context(tc.tile_pool(name="consts", bufs=1))
        big = ctx.enter_context(tc.tile_pool(name="big", bufs=1))
        expp = ctx.enter_context(tc.tile_pool(name="expp", bufs=4))
        resp = ctx.enter_context(tc.tile_pool(name="resp", bufs=2))
        # PSUM 8 banks: scp 3x[128,1024]f32 (6 banks) shared by score pairs
        # AND the projection chunks; T-pool 1x[128,1024] (2 banks) holds
        # pv (cols 0:512), out-proj cc0 (512:1024), cc1 (0:512 after res).
        scp = ctx.enter_context(tc.tile_pool(name="scp", bufs=3, space="PSUM"))
        tp = ctx.enter_context(tc.tile_pool(name="tp", bufs=1, space="PSUM"))

        # ---- weights / constants ----
        w_sb = consts.tile([128, 2, 3 * DK], f16)
        for idx, w_dram in enumerate((wq, wk, wv)):
            nc.gpsimd.dma_start(
                out=w_sb[:, :, idx * DK : (idx + 1) * DK],
                in_=w_dram.rearrange("(c p) d -> p c d", p=128),
            )

        def wslice(idx, cc):
            return w_sb[:, cc, idx * DK : (idx + 1) * DK]

        wo_sb = consts.tile([DK, C], f16)
        nc.gpsimd.dma_start(out=wo_sb, in_=wo[:, :])
        b_sb = consts.tile([DK, 2], f32)
        nc.gpsimd.dma_start(out=b_sb, in_=bqk[:, :])
        ebias_sb = consts.tile([128, 1], f32)
        nc.vector.memset(ebias_sb, -3.0)

        # ---- x in SBUF ----
        x_sb = big.tile([128, 2, SP], f16)
        nc.vector.memset(x_sb[:, :, S:SP], 0.0)
        nc.sync.dma_start(
            out=x_sb[:, :, :S], in_=xT.rearrange("(c p) s -> p c s", p=128)
        )

        # ---- persistent big tiles ----
        qT_sb = big.tile([DK, SP], f16)
        kT_sb = big.tile([DK, SP], f16)
        # v8: [j-in-tile, group, plane(u), 128]: cols 0:64 v, 64 ones, 65+ 0
        v8_sb = big.tile([128, NG, 2, 128], f8)
        nc.vector.memset(v8_sb, 0.0)
        nc.vector.memset(v8_sb[:, :, :, DK : DK + 1], 1.0)
        # last j-tile (group 10, plane 1) has only SVALID_LAST valid rows;
        # partition-base-56 memset is illegal, so DMA a host mask instead
        nc.gpsimd.dma_start(
            out=v8_sb[:, NG - 1, 1, DK : DK + 1], in_=ones_tail[:, :]
        )
        out_sb = big.tile([128, 2, S], f16)

        Identity = mybir.ActivationFunctionType.Identity
        QBLOCKS = [(0, 1024), (1024, 1024), (2048, 768)]

        # ---- q/k projection chunk (1024-col blocks) ----
        def qk_chunk(qb, widx, dst, bias, eng):
            off, w = QBLOCKS[qb]
            ps = scp.tile([128, 1024], f32, tag="sc", name="psqk")
            for half in range(0, w, 512):
                hw_ = min(512, w - half)
                for cc in range(2):
                    nc.tensor.matmul(
                        ps[:DK, half : half + hw_],
                        lhsT=wslice(widx, cc),
                        rhs=x_sb[:, cc, off + half : off + half + hw_],
                        start=(cc == 0),
                        stop=(cc == 1),
                    )
            if eng is nc.scalar:
                nc.scalar.activation(
                    out=dst[:, off : off + w],
                    in_=ps[:DK, :w],
                    func=Identity,
                    bias=bias,
                )
            else:
                eng.tensor_scalar(
                    dst[:, off : off + w], ps[:DK, :w], bias, None,
                    mybir.AluOpType.add,
                )

        # ---- v projection chunk: 4 j-tiles -> v8 groups 2c, 2c+1 ----
        def v_chunk(c, eng):
            nt = min(4, NT - 4 * c)
            ps = scp.tile([128, 1024], f32, tag="sc", name="psv")
            pv4 = ps[:, : nt * DK].rearrange("p (t d) -> p t d", t=nt)
            for ti in range(nt):
                t = 4 * c + ti
                for cc in range(2):
                    nc.tensor.matmul(
                        pv4[:, ti, :],
                        lhsT=x_sb[:, cc, t * 128 : (t + 1) * 128],
                        rhs=wslice(2, cc),
                        start=(cc == 0),
                        stop=(cc == 1),
                    )
            dst = v8_sb[:, 2 * c : 2 * c + (nt + 1) // 2, :, :DK]
            eng.tensor_copy(dst, pv4)

        # phase A: k fully, q block 0, v chunk 0
        qk_chunk(0, 1, kT_sb, b_sb[:, 1:2], nc.scalar)
        qk_chunk(1, 1, kT_sb, b_sb[:, 1:2], nc.vector)
        qk_chunk(2, 1, kT_sb, b_sb[:, 1:2], nc.scalar)
        qk_chunk(0, 0, qT_sb, b_sb[:, 0:1], nc.vector)
        v_chunk(0, nc.vector)

        # chunks interleaved into iblock 0 (key: g -> emit fn)
        ib0_chunks = {
            1: lambda: v_chunk(1, nc.vector),
            2: lambda: qk_chunk(1, 0, qT_sb, b_sb[:, 0:1], nc.scalar),
            3: lambda: v_chunk(2, nc.vector),
            4: lambda: qk_chunk(2, 0, qT_sb, b_sb[:, 0:1], nc.scalar),
            5: lambda: v_chunk(3, nc.vector),
            7: lambda: v_chunk(4, nc.vector),
            9: lambda: v_chunk(5, nc.vector),
        }

        # ---- main attention loop ----
        # PV queue lag 3 and carried across iblock boundaries: the PE queue
        # is in-order, so a PV emitted right after its exp stalls the score
        # matmuls behind it; with lag 3 the exp has long completed.
        def emit_pv(pv, pex, pg, iw):
            nc.tensor.matmul(
                pv[:, :iw],
                lhsT=v8_sb[:, pg, :, :],
                rhs=pex[:, :, :iw],
                start=(pg == 0),
                stop=(pg == NG - 1),
                perf_mode=DR,
            )

        def res_and_lsum(ibi, pv, ioff, iw):
            res_sb = resp.tile([DK + 1, 512], f16, tag="res", name="res_sb")
            if ibi % 2 == 0:
                nc.scalar.copy(res_sb[:, :iw], pv[: DK + 1, :iw])
            else:
                nc.vector.tensor_copy(res_sb[:, :iw], pv[: DK + 1, :iw])
            nc.sync.dma_start(
                out=lsum[0:1, ioff : ioff + iw], in_=res_sb[DK : DK + 1, :iw]
            )
            return res_sb

        def make_tail(ibi, T, res_sb, ioff, iw):
            def tail():
                po = T.rearrange("p (b w) -> p b w", b=2)[:, :, :iw]
                for cc in range(2):
                    # cc0 reuses pv's half (waits res copy), cc1 the other
                    nc.tensor.matmul(
                        po[:, cc, :],
                        lhsT=wo_sb[:, cc * 128 : (cc + 1) * 128],
                        rhs=res_sb[:DK, :iw],
                        start=True,
                        stop=True,
                    )
                dsto = out_sb[:, :, ioff : ioff + iw]
                if ibi % 2 == 0:
                    nc.vector.tensor_copy(dsto, po)
                else:
                    nc.scalar.copy(dsto, po)

            return tail

        pvq = []  # [(ex, g, ibi)]
        state = {"T": None, "prev": None}  # prev: (ibi, pv, ioff, iw)

        def pop_pv():
            ex, g, ibi_ = pvq.pop(0)
            ioff_, iw_ = IBLOCKS[ibi_]
            emit_pv(state["T"][:, 0:512], ex, g, iw_)

        for ibi, (ioff, iw) in enumerate(IBLOCKS):
            for g in range(NG):
                if ibi == 0 and g in ib0_chunks:
                    ib0_chunks[g]()
                sc = scp.tile([128, 1024], f32, tag="sc", name="sc")
                sc3 = sc.rearrange("p (b w) -> p b w", b=2)[:, :, :iw]
                for u in range(2):
                    t = 2 * g + u
                    nc.tensor.matmul(
                        sc3[:, u, :],
                        lhsT=kT_sb[:, t * 128 : (t + 1) * 128],
                        rhs=qT_sb[:, ioff : ioff + iw],
                        start=True,
                        stop=True,
                    )
                ex = expp.tile([128, 2, 512], f8, tag="ex", name="ex")
                # exp halves: ScalarE does plane 0, DVE plane 1 concurrently
                nc.scalar.activation(
                    out=ex[:, 0, :iw],
                    in_=sc3[:, 0, :],
                    func=Exp,
                    bias=ebias_sb,
                    scale=0.125,
                )
                nc.vector.tensor_scalar(
                    ex[:, 1, :iw].bitcast(u8), sc3[:, 1, :],
                    SCHRAU_SCALE, SCHRAU_BIAS, Mult, Add,
                )
                pvq.append((ex, g, ibi))
                if g == 2 and ibi > 0:
                    pop_pv()  # pops PV(10, prev); pv(prev) now complete
                    pibi, ppv, pioff, piw = state["prev"]
                    state["res"] = res_and_lsum(pibi, ppv, pioff, piw)
                elif g == 3:
                    if ibi > 0:
                        pibi, ppv, pioff, piw = state["prev"]
                        make_tail(pibi, state["T"], state["res"], pioff, piw)()
                    state["T"] = tp.tile([128, 1024], f32, tag="T", name="T")
                    pop_pv()  # first PV of this iblock, into the new T
                elif len(pvq) > 3:
                    pop_pv()
            state["prev"] = (ibi, state["T"][:, 0:512], ioff, iw)
        while pvq:
            pop_pv()
        pibi, ppv, pioff, piw = state["prev"]
        res_sb = res_and_lsum(pibi, ppv, pioff, piw)
        make_tail(pibi, state["T"], res_sb, pioff, piw)()
        nc.sync.dma_start(
            out=out.rearrange("(c p) s -> p c s", p=128), in_=out_sb
        )

    nc.compile()
    return nc


def _get_nc():
    global _NC
    if _NC is None:
        _NC = _build()
    return _NC


def _ones_tail():
    import ml_dtypes

    m = np.zeros((128, 1), dtype=ml_dtypes.float8_e4m3)
    m[:SVALID_LAST] = 1.0
    return m


def _make_in_maps(inputs):
    x = np.asarray(inputs["x"], dtype=np.float32)
    w_proj = np.asarray(inputs["w_proj"], dtype=np.float32)
    b_proj = np.asarray(inputs["b_proj"], dtype=np.float32)
    w_out = np.asarray(inputs["w_out"], dtype=np.float32)
    in_maps = []
    for core in range(8):
        b, h = divmod(core, H)
        base = h * 3 * DK
        in_maps.append(
            {
                "xT": np.ascontiguousarray(x[b].reshape(C, S).astype(np.float16)),
                "wq": np.ascontiguousarray(
                    w_proj[:, base : base + DK].astype(np.float16)
                ),
                "wk": np.ascontiguousarray(
                    w_proj[:, base + DK : base + 2 * DK].astype(np.float16)
                ),
                "wv": np.ascontiguousarray(
                    w_proj[:, base + 2 * DK : base + 3 * DK].astype(np.float16)
                ),
                "bqk": np.ascontiguousarray(
                    np.stack(
                        [
                            b_proj[base : base + DK],
                            b_proj[base + DK : base + 2 * DK],
                        ],
                        axis=1,
                    ).astype(np.float32)
                ),
                "wo": np.ascontiguousarray(
                    w_out[h * DK : (h + 1) * DK, :].astype(np.float16)
                ),
                "ones_tail": _ones_tail(),
            }
        )
    return in_maps


def kernel(x, w_proj, b_proj, w_out, b_out):
    from concourse.bass_utils import run_bass_kernel_spmd

    x = np.asarray(x, dtype=np.float32)
    w_proj = np.asarray(w_proj, dtype=np.float32)
    b_proj = np.asarray(b_proj, dtype=np.float32)
    w_out = np.asarray(w_out, dtype=np.float32)
    b_out = np.asarray(b_out, dtype=np.float32)

    B = x.shape[0]
    nc = _get_nc()
    in_maps = _make_in_maps(
        {"x": x, "w_proj": w_proj, "b_proj": b_proj, "w_out": w_out, "b_out": b_out}
    )
    res = run_bass_kernel_spmd(nc, in_maps, list(range(8)))

    outs = np.zeros((B, C, S), dtype=np.float32)
    for b in range(B):
        acc = x[b].reshape(C, S).astype(np.float32) + b_out[:, None]
        for h in range(H):
            core = b * H + h
            dev_o = res.results[core]["out"].astype(np.float32)  # [C, S] unnorm
            l = res.results[core]["lsum"].astype(np.float32)  # [1, S]
            bv = b_proj[h * 3 * DK + 2 * DK : h * 3 * DK + 3 * DK]
            corr = bv @ w_out[h * DK : (h + 1) * DK, :]  # [C]
            acc = acc + dev_o / l + corr[:, None]
        outs[b] = acc
    return outs.reshape(B, C, 14, 14, 14)
